# revision 16
# baseline (speedup 1.0000x reference)
"""AdditiveAttention (Bahdanau) Trainium2 kernel, SPMD over 8 NeuronCores.

Reference computation (B=32, T=2048, Q=K=H=1024):
    trans_q  = queries @ W_q                        (B, 1, H)
    trans_k  = keys @ W_k                           (B, T, H)   <-- dominant matmul
    features = tanh(trans_q + trans_k)              (B, T, H)
    scores   = features @ w_v                       (B, T, 1)
    scores   = where(pad_mask, -inf, scores)
    attn     = softmax(scores, axis=1)              (B, T, 1)
    context  = einsum('btk,bto->bko', keys, attn)   (B, K, 1)
    returns (attn, context)

Sharding: data-parallel over batch, 4 batches per core, weights replicated.

Device layout (per core, per batch):
  - keysT (bf16) in SBUF as [128k x (8kc*2048t)]; main matmul computes
    trans_k^T tiles [128h, 512t] = W_k[kc,hc]^T.T @ keysT (accumulate 8 kc).
  - ScalarE evicts PSUM with fused tanh(x + trans_q[h]) -> features bf16.
  - scores^T [1, 512t] = w_v[hc]^T.T @ features (accumulate 8 hc).
  - softmax over the free dim on partition 0 (mask added as -1e9 bias).
  - attn broadcast to 128 partitions with a ones[1,128] PE matmul,
    context[k] via fused DVE multiply+reduce over keysT rows.

All matmul inputs are bf16 (fp32 accumulation); softmax is fp32. Host-side
prep does layout/transpose/dtype only, plus the (negligible) trans_q GEMM is
done on-device too.
"""

import numpy as np
import ml_dtypes

B, T, QD, KD, H = 32, 2048, 1024, 1024, 1024
NCORES = 8
BPC = B // NCORES  # batches per core
KC = 8  # contraction chunks (K=1024 / 128)
HC = 8  # hidden chunks (H=1024 / 128)
TT = 4  # t tiles of 512 (T=2048)
TN = 512

BF16 = ml_dtypes.bfloat16


def _apply_tile_patch():
    """This walrus build rejects >2 sync waits on one instruction; re-emit the
    Tile tail-drain waits as individual single-wait instructions on SP."""
    from concourse.tile import TileContext
    from concourse.vector_clock import ScopedClock

    if getattr(TileContext, "_drain_patch_applied", False):
        return

    def _patched_drain_and_barrier(self, tick_clock, wait_clock):
        nc = self.nc
        nop = nc.sync.nop(nofuse=True)
        wait_clock.add_sem_waits(nop.ins, ScopedClock({None: tick_clock.global_clock}))
        waits = list(nop.ins.sync_info.on_wait) if nop.ins.sync_info else []
        nop.ins.sync_info = None
        sem_by_num = {s.num: s for s in self.sems.allocated().values()}
        for w in waits:
            nc.sync.wait_ge(sem_by_num[w.id], w.wait_value)
        nc.sync.drain()
        nc.all_engine_barrier()
        popped = nc._tile_sem_poison_stack.pop()
        assert popped is self._sem_poison
        nc.clear_and_free_semaphores(list(self.sems.allocated().values()))
        nc.all_engine_barrier()

    TileContext._drain_and_barrier = _patched_drain_and_barrier
    TileContext._drain_patch_applied = True


def build_nc():
    import concourse.mybir as mybir
    from concourse import bacc
    from concourse.tile import TileContext

    _apply_tile_patch()

    f32 = mybir.dt.float32
    bf16 = mybir.dt.bfloat16
    AF = mybir.ActivationFunctionType
    ALU = mybir.AluOpType
    AX = mybir.AxisListType

    nc = bacc.Bacc()

    keysT_p = nc.declare_dram_parameter("keysT", [BPC, KC, 128, T], bf16, isOutput=False)
    wk_p = nc.declare_dram_parameter("wk", [KC, 128, H], bf16, isOutput=False)
    wq_p = nc.declare_dram_parameter("wq", [KC, 128, H], bf16, isOutput=False)
    qt_p = nc.declare_dram_parameter("qt", [KC, 128, BPC], bf16, isOutput=False)
    wv_p = nc.declare_dram_parameter("wv", [128, HC], bf16, isOutput=False)
    maskb_p = nc.declare_dram_parameter("maskb", [BPC, T], bf16, isOutput=False)
    attn_p = nc.declare_dram_parameter("attn", [BPC, T], f32, isOutput=True)
    ctx_p = nc.declare_dram_parameter("ctx", [BPC, 128, KC], f32, isOutput=True)

    with TileContext(nc) as tc:
        with (
            tc.tile_pool(name="const", bufs=1) as const,
            tc.tile_pool(name="wk", bufs=KC) as wkpool,
            tc.tile_pool(name="wqs", bufs=KC) as wqpool,
            tc.tile_pool(name="keys", bufs=2 * KC) as kpool,
            tc.tile_pool(name="feat", bufs=10) as fpool,
            tc.tile_pool(name="p0", bufs=2) as p0pool,
            tc.tile_pool(name="bc", bufs=3) as bcpool,
            tc.tile_pool(name="psum", bufs=1, space="PSUM") as psum,
        ):
            # ---- constants (small DMAs go on the ACT HWDGE ring so the SP
            # ring stays clear for the big weight/key transfers) ----
            ones_sb = const.tile([1, 128], bf16, tag="ones")
            nc.vector.memset(ones_sb[:], 1.0)
            ones_f = const.tile([1, 128], f32, tag="onesf")
            nc.vector.memset(ones_f[:], 1.0)
            wv_sb = const.tile([128, HC], bf16, tag="wv")
            nc.scalar.dma_start(out=wv_sb[:], in_=wv_p[:])
            qt_sb = const.tile([128, KC * BPC], bf16, tag="qt")
            nc.scalar.dma_start(
                out=qt_sb[:].rearrange("p (kc b) -> p kc b", kc=KC),
                in_=qt_p[:].rearrange("kc p b -> p kc b"),
            )
            q_sb = const.tile([128, HC * BPC], f32, tag="qsb")

            # W_k as one tile per k-chunk so matmuls depend on 256KB, not 2MB
            wk_sb = []
            kt_tiles = {}

            def load_kt(b):
                tiles = []
                for kc in range(KC):
                    t = kpool.tile([128, T], bf16, tag="kt")
                    nc.sync.dma_start(out=t[:], in_=keysT_p[b, kc])
                    tiles.append(t)
                kt_tiles[b] = tiles

            for kc in range(KC):
                w = wkpool.tile([128, H], bf16, tag="wk")
                nc.sync.dma_start(out=w[:], in_=wk_p[kc])
                wk_sb.append(w)
            load_kt(0)

            # ---- trans_q^T = W_q^T q  ->  q_sb[:, hc*BPC + b] ----
            wq_sb = []
            for kc in range(KC):
                w = wqpool.tile([128, H], bf16, tag="wqt")
                nc.sync.dma_start(out=w[:], in_=wq_p[kc])
                wq_sb.append(w)
            for hc in range(HC):
                pq = psum.tile([128, BPC], f32, tag="ps", bufs=2)
                for kc in range(KC):
                    nc.tensor.matmul(
                        pq[:],
                        lhsT=wq_sb[kc][:, hc * 128:(hc + 1) * 128],
                        rhs=qt_sb[:, kc * BPC:(kc + 1) * BPC],
                        start=(kc == 0),
                        stop=(kc == KC - 1),
                    )
                nc.scalar.copy(q_sb[:, hc * BPC:(hc + 1) * BPC], pq[:])

            # ---- per-batch pipeline; softmax has no max-subtraction (scores
            # are O(+-4) so exp is fp32-safe), which lets exp / broadcast /
            # context run per 512-wide t-tile, overlapped with the matmuls ----
            state = {}

            def emit_tt(b, tt):
                """main matmuls + tanh + scores(+mask) + exp for one t-tile"""
                kt, mask_sb, ex16, se4, _ = state[b]
                fts = []
                for hc in range(HC):
                    pf = psum.tile([128, TN], f32, tag="pf", bufs=2)
                    for kc in range(KC):
                        nc.tensor.matmul(
                            pf[:],
                            lhsT=wk_sb[kc][:, hc * 128:(hc + 1) * 128],
                            rhs=kt[kc][:, tt * TN:(tt + 1) * TN],
                            start=(kc == 0),
                            stop=(kc == KC - 1),
                        )
                    ft = fpool.tile([128, TN], bf16, tag="ft")
                    nc.scalar.activation(
                        ft[:], pf[:], AF.Tanh,
                        bias=q_sb[:, hc * BPC + b: hc * BPC + b + 1],
                    )
                    fts.append(ft)
                ps = psum.tile([1, TN], f32, tag="ps", bufs=2)
                for hc in range(HC):
                    nc.tensor.matmul(
                        ps[:],
                        lhsT=wv_sb[:, hc: hc + 1],
                        rhs=fts[hc][:],
                        start=(hc == 0),
                        stop=False,
                    )
                # rank-1 update adds the additive mask row (-1e9 on padded)
                nc.tensor.matmul(
                    ps[:],
                    lhsT=ones_sb[:, 0:1],
                    rhs=mask_sb[:, tt * TN:(tt + 1) * TN],
                    start=False,
                    stop=True,
                )
                # exp straight from PSUM; bf16 numerators + f32 partial sum
                nc.scalar.activation(
                    ex16[:, tt * TN:(tt + 1) * TN], ps[:], AF.Exp,
                    accum_out=se4[:, tt: tt + 1],
                )

            def emit_ctx_tt(b, tt):
                """broadcast exp(tt) over partitions, ctx partial sums"""
                kt, mask_sb, ex16, se4, _ = state[b]
                pb = psum.tile([128, TN], f32, tag="pb", bufs=2)
                nc.tensor.matmul(
                    pb[:],
                    lhsT=ones_sb[:],
                    rhs=ex16[:, tt * TN:(tt + 1) * TN],
                    start=True,
                    stop=True,
                )
                bc16 = bcpool.tile([128, TN], bf16, tag="bc")
                nc.scalar.copy(bc16[:], pb[:])
                scr = bcpool.tile([128, KC * TN], bf16, tag="scr", bufs=2)
                for kc in range(KC):
                    nc.vector.tensor_mul(
                        scr[:, kc * TN:(kc + 1) * TN],
                        kt[kc][:, tt * TN:(tt + 1) * TN],
                        bc16[:],
                    )
                ctxp = bcpool.tile([128, KC], f32, tag="ctxp", bufs=5)
                nc.vector.reduce_sum(
                    ctxp[:], scr[:].rearrange("p (kc t) -> p kc t", kc=KC), axis=AX.X
                )
                state[b][4].append(ctxp)

            def emit_finish(b):
                """normalize: attn = ex/sum out, ctx = (sum of partials)/sum"""
                kt, mask_sb, ex16, se4, ctxps = state.pop(b)
                se = p0pool.tile([1, 1], f32, tag="se")
                nc.vector.reduce_sum(se[:], se4[:], axis=AX.X)
                rcp = p0pool.tile([1, 1], f32, tag="rcp")
                nc.vector.reciprocal(rcp[:], se[:])
                attn_f = p0pool.tile([1, T], f32, tag="attnf")
                nc.vector.tensor_scalar_mul(attn_f[:], ex16[:], rcp[:])
                nc.scalar.dma_start(out=attn_p[b].unsqueeze(0), in_=attn_f[:])
                # rcp broadcast across partitions via fp32 rank-1 matmul
                prc = psum.tile([128, 1], f32, tag="pb", bufs=2)
                nc.tensor.matmul(prc[:], lhsT=ones_f[:], rhs=rcp[:], start=True, stop=True)
                rcp_bc = bcpool.tile([128, 1], f32, tag="rcpbc", bufs=2)
                nc.scalar.copy(rcp_bc[:], prc[:])
                acc = bcpool.tile([128, KC], f32, tag="ctxacc", bufs=2)
                nc.vector.tensor_add(acc[:], ctxps[0][:], ctxps[1][:])
                nc.vector.tensor_add(acc[:], acc[:], ctxps[2][:])
                nc.vector.tensor_add(acc[:], acc[:], ctxps[3][:])
                ctx_sb = bcpool.tile([128, KC], f32, tag="ctx", bufs=2)
                nc.vector.tensor_scalar_mul(ctx_sb[:], acc[:], rcp_bc[:])
                nc.scalar.dma_start(out=ctx_p[b], in_=ctx_sb[:])

            def start_batch(b):
                mask_sb = p0pool.tile([1, T], bf16, tag="mask")
                nc.scalar.dma_start(out=mask_sb[:], in_=maskb_p[b].unsqueeze(0))
                ex16 = p0pool.tile([1, T], bf16, tag="ex16")
                se4 = p0pool.tile([1, TT], f32, tag="se4")
                state[b] = [kt_tiles.pop(b), mask_sb, ex16, se4, []]

            start_batch(0)
            for b in range(BPC):
                if b + 1 < BPC:
                    load_kt(b + 1)
                for tt in range(TT):
                    emit_tt(b, tt)
                    if tt > 0:
                        emit_ctx_tt(b, tt - 1)
                if b + 1 < BPC:
                    start_batch(b + 1)
                emit_ctx_tt(b, TT - 1)
                emit_finish(b)

    nc.compile()
    return nc


def _prep_in_maps(queries, keys, pad_mask, W_q, W_k, w_v):
    """Host-side sharding + layout prep (dtype casts, transposes, mask bias)."""
    wk16 = np.ascontiguousarray(W_k.astype(BF16)).reshape(KC, 128, H)
    wq16 = np.ascontiguousarray(W_q.astype(BF16)).reshape(KC, 128, H)
    wv16 = np.ascontiguousarray(w_v[:, 0].reshape(HC, 128).T.astype(BF16))
    maskb = np.where(pad_mask[:, :, 0], np.float32(-1e9), np.float32(0.0)).astype(BF16)

    in_maps = []
    for c in range(NCORES):
        b0 = c * BPC
        kT = np.ascontiguousarray(
            keys[b0:b0 + BPC].transpose(0, 2, 1)
        ).astype(BF16).reshape(BPC, KC, 128, T)
        qt = np.ascontiguousarray(
            queries[b0:b0 + BPC, 0, :].T.reshape(KC, 128, BPC)
        ).astype(BF16)
        in_maps.append({
            "keysT": kT,
            "wk": wk16,
            "wq": wq16,
            "qt": qt,
            "wv": wv16,
            "maskb": np.ascontiguousarray(maskb[b0:b0 + BPC]),
        })
    return in_maps


def run(inputs, trace=False):
    """Build, compile, run on 8 cores. Returns ((attn, context), exec_time_ns)."""
    from concourse.bass_utils import run_bass_kernel_spmd

    in_maps = _prep_in_maps(**inputs)
    nc = build_nc()
    res = run_bass_kernel_spmd(nc, in_maps, list(range(NCORES)), trace=trace)

    attn = np.empty((B, T, 1), dtype=np.float32)
    context = np.empty((B, KD, 1), dtype=np.float32)
    for c in range(NCORES):
        b0 = c * BPC
        attn[b0:b0 + BPC, :, 0] = res.results[c]["attn"]
        # ctx result layout [b, p, kc] -> k = kc*128 + p
        context[b0:b0 + BPC, :, 0] = (
            res.results[c]["ctx"].transpose(0, 2, 1).reshape(BPC, KD)
        )
    return (attn, context), res.exec_time_ns


def kernel(queries, keys, pad_mask, W_q, W_k, w_v):
    (attn, context), _ = run(
        dict(queries=np.asarray(queries), keys=np.asarray(keys),
             pad_mask=np.asarray(pad_mask), W_q=np.asarray(W_q),
             W_k=np.asarray(W_k), w_v=np.asarray(w_v))
    )
    return attn, context


# revision 21
# speedup vs baseline: 1.2232x; 1.2232x over previous
"""AdditiveAttention (Bahdanau) Trainium2 kernel, SPMD over 8 NeuronCores.

Reference computation (B=32, T=2048, Q=K=H=1024):
    trans_q  = queries @ W_q                        (B, 1, H)
    trans_k  = keys @ W_k                           (B, T, H)   <-- dominant matmul
    features = tanh(trans_q + trans_k)              (B, T, H)
    scores   = features @ w_v                       (B, T, 1)
    scores   = where(pad_mask, -inf, scores)
    attn     = softmax(scores, axis=1)              (B, T, 1)
    context  = einsum('btk,bto->bko', keys, attn)   (B, K, 1)
    returns (attn, context)

Sharding: data-parallel over batch, 4 batches per core, weights replicated.

Device layout (per core, per batch):
  - keysT (bf16) in SBUF as [128k x (8kc*2048t)]; main matmul computes
    trans_k^T tiles [128h, 512t] = W_k[kc,hc]^T.T @ keysT (accumulate 8 kc).
  - ScalarE evicts PSUM with fused tanh(x + trans_q[h]) -> features bf16.
  - scores^T [1, 512t] = w_v[hc]^T.T @ features (accumulate 8 hc).
  - softmax over the free dim on partition 0 (mask added as -1e9 bias).
  - attn broadcast to 128 partitions with a ones[1,128] PE matmul,
    context[k] via fused DVE multiply+reduce over keysT rows.

All matmul inputs are bf16 (fp32 accumulation); softmax is fp32. Host-side
prep does layout/transpose/dtype only, plus the (negligible) trans_q GEMM is
done on-device too.
"""

import numpy as np
import ml_dtypes

B, T, QD, KD, H = 32, 2048, 1024, 1024, 1024
NCORES = 8
BPC = B // NCORES  # batches per core
KC = 8  # contraction chunks (K=1024 / 128)
HC = 8  # hidden chunks (H=1024 / 128)
TT = 4  # t tiles of 512 (T=2048)
TN = 512

BF16 = ml_dtypes.bfloat16


def _apply_tile_patch():
    """This walrus build rejects >2 sync waits on one instruction; re-emit the
    Tile tail-drain waits as individual single-wait instructions on SP."""
    from concourse.tile import TileContext
    from concourse.vector_clock import ScopedClock

    if getattr(TileContext, "_drain_patch_applied", False):
        return

    def _patched_drain_and_barrier(self, tick_clock, wait_clock):
        nc = self.nc
        nop = nc.sync.nop(nofuse=True)
        wait_clock.add_sem_waits(nop.ins, ScopedClock({None: tick_clock.global_clock}))
        waits = list(nop.ins.sync_info.on_wait) if nop.ins.sync_info else []
        nop.ins.sync_info = None
        sem_by_num = {s.num: s for s in self.sems.allocated().values()}
        for w in waits:
            nc.sync.wait_ge(sem_by_num[w.id], w.wait_value)
        nc.sync.drain()
        nc.all_engine_barrier()
        popped = nc._tile_sem_poison_stack.pop()
        assert popped is self._sem_poison
        nc.clear_and_free_semaphores(list(self.sems.allocated().values()))
        nc.all_engine_barrier()

    TileContext._drain_and_barrier = _patched_drain_and_barrier
    TileContext._drain_patch_applied = True


def _apply_ldw_opt_patch():
    """The stock walrus invocation passes --enable-ldw-opt=false, which
    serializes every LDWEIGHTS with its MATMUL (~+150ns per 128x128-weight
    matmul). Enable the background-weight-buffer overlap."""
    from concourse import bass_utils as _bu

    if getattr(_bu, "_ldw_opt_patched", False):
        return
    _orig = _bu.run_command

    def _patched(cmd, **kw):
        if isinstance(cmd, list):
            cmd = ["--enable-ldw-opt=true" if c == "--enable-ldw-opt=false" else c
                   for c in cmd]
        return _orig(cmd, **kw)

    _bu.run_command = _patched
    _bu._ldw_opt_patched = True


def build_nc():
    import concourse.mybir as mybir
    from concourse import bacc
    from concourse.tile import TileContext

    _apply_tile_patch()
    # NOTE: --enable-ldw-opt=true crashes this walrus build (visitInstLdweights)
    # and measurement shows self-loading matmuls already overlap their weight
    # load, so no ldw patch is applied.

    f32 = mybir.dt.float32
    bf16 = mybir.dt.bfloat16
    AF = mybir.ActivationFunctionType
    ALU = mybir.AluOpType
    AX = mybir.AxisListType

    nc = bacc.Bacc()

    keysT_p = nc.declare_dram_parameter("keysT", [BPC, KC, 128, T], bf16, isOutput=False)
    wk_p = nc.declare_dram_parameter("wk", [KC, 128, H], bf16, isOutput=False)
    wq_p = nc.declare_dram_parameter("wq", [KC, 128, H], bf16, isOutput=False)
    qt_p = nc.declare_dram_parameter("qt", [KC, 128, BPC], bf16, isOutput=False)
    wv_p = nc.declare_dram_parameter("wv", [128, HC], bf16, isOutput=False)
    maskb_p = nc.declare_dram_parameter("maskb", [BPC, T], bf16, isOutput=False)
    attn_p = nc.declare_dram_parameter("attn", [BPC, T], f32, isOutput=True)
    ctx_p = nc.declare_dram_parameter("ctx", [BPC, 128, KC], f32, isOutput=True)

    with TileContext(nc) as tc:
        with (
            tc.tile_pool(name="const", bufs=1) as const,
            tc.tile_pool(name="wk", bufs=KC) as wkpool,
            tc.tile_pool(name="wqs", bufs=KC) as wqpool,
            tc.tile_pool(name="keys", bufs=2 * KC) as kpool,
            tc.tile_pool(name="feat", bufs=10) as fpool,
            tc.tile_pool(name="p0", bufs=2) as p0pool,
            tc.tile_pool(name="bc", bufs=3) as bcpool,
            tc.tile_pool(name="psum", bufs=1, space="PSUM") as psum,
        ):
            # ---- constants (small DMAs go on the ACT HWDGE ring so the SP
            # ring stays clear for the big weight/key transfers) ----
            ones_sb = const.tile([1, 128], bf16, tag="ones")
            nc.vector.memset(ones_sb[:], 1.0)
            ones_f = const.tile([1, 128], f32, tag="onesf")
            nc.vector.memset(ones_f[:], 1.0)
            wv_sb = const.tile([128, HC], bf16, tag="wv")
            nc.scalar.dma_start(out=wv_sb[:], in_=wv_p[:])
            qt_sb = const.tile([128, KC * BPC], bf16, tag="qt")
            nc.scalar.dma_start(
                out=qt_sb[:].rearrange("p (kc b) -> p kc b", kc=KC),
                in_=qt_p[:].rearrange("kc p b -> p kc b"),
            )
            q_sb = const.tile([128, HC * BPC], f32, tag="qsb")

            # W_k as one tile per k-chunk so matmuls depend on 256KB, not 2MB;
            # interleave wk/kt DMAs so the kc-ordered consumers start earliest.
            # W_q rides the ACT HWDGE ring so it doesn't delay batch-0 keys.
            wk_sb = []
            wq_sb = []
            kt_tiles = {}

            def load_kt(b):
                tiles = []
                for kc in range(KC):
                    t = kpool.tile([128, T], bf16, tag="kt")
                    nc.sync.dma_start(out=t[:], in_=keysT_p[b, kc])
                    tiles.append(t)
                kt_tiles[b] = tiles

            for kc in range(KC):
                w = wkpool.tile([128, H], bf16, tag="wk")
                nc.sync.dma_start(out=w[:], in_=wk_p[kc])
                wk_sb.append(w)
                t0 = kpool.tile([128, T], bf16, tag="kt")
                nc.sync.dma_start(out=t0[:], in_=keysT_p[0, kc])
                kt_tiles.setdefault(0, []).append(t0)
                wq = wqpool.tile([128, H], bf16, tag="wqt")
                nc.scalar.dma_start(out=wq[:], in_=wq_p[kc])
                wq_sb.append(wq)
            for hc in range(HC):
                pq = psum.tile([128, BPC], f32, tag="ps", bufs=2)
                for kc in range(KC):
                    nc.tensor.matmul(
                        pq[:],
                        lhsT=wq_sb[kc][:, hc * 128:(hc + 1) * 128],
                        rhs=qt_sb[:, kc * BPC:(kc + 1) * BPC],
                        start=(kc == 0),
                        stop=(kc == KC - 1),
                    )
                nc.scalar.copy(q_sb[:, hc * BPC:(hc + 1) * BPC], pq[:])

            # ---- per-batch pipeline; softmax has no max-subtraction (scores
            # are O(+-4) so exp is fp32-safe), which lets exp / broadcast /
            # context run per 512-wide t-tile, overlapped with the matmuls ----
            state = {}

            def emit_tt(b, tt):
                """main matmuls + tanh + scores(+mask) + exp for one t-tile"""
                kt, mask_sb, ex16, se4, _ = state[b]
                fts = []
                for hc in range(HC):
                    pf = psum.tile([128, TN], f32, tag="pf", bufs=2)
                    for kc in range(KC):
                        nc.tensor.matmul(
                            pf[:],
                            lhsT=wk_sb[kc][:, hc * 128:(hc + 1) * 128],
                            rhs=kt[kc][:, tt * TN:(tt + 1) * TN],
                            start=(kc == 0),
                            stop=(kc == KC - 1),
                        )
                    ft = fpool.tile([128, TN], bf16, tag="ft")
                    nc.scalar.activation(
                        ft[:], pf[:], AF.Tanh,
                        bias=q_sb[:, hc * BPC + b: hc * BPC + b + 1],
                    )
                    fts.append(ft)
                ps = psum.tile([1, TN], f32, tag="ps", bufs=2)
                for hc in range(HC):
                    nc.tensor.matmul(
                        ps[:],
                        lhsT=wv_sb[:, hc: hc + 1],
                        rhs=fts[hc][:],
                        start=(hc == 0),
                        stop=False,
                    )
                # rank-1 update adds the additive mask row (-1e9 on padded)
                nc.tensor.matmul(
                    ps[:],
                    lhsT=ones_sb[:, 0:1],
                    rhs=mask_sb[:, tt * TN:(tt + 1) * TN],
                    start=False,
                    stop=True,
                )
                # exp straight from PSUM; bf16 numerators + f32 partial sum
                nc.scalar.activation(
                    ex16[:, tt * TN:(tt + 1) * TN], ps[:], AF.Exp,
                    accum_out=se4[:, tt: tt + 1],
                )

            def emit_ctx_tt(b, tt):
                """broadcast exp(tt) over partitions, ctx partial sums"""
                kt, mask_sb, ex16, se4, _ = state[b]
                pb = psum.tile([128, TN], f32, tag="pb", bufs=2)
                nc.tensor.matmul(
                    pb[:],
                    lhsT=ones_sb[:],
                    rhs=ex16[:, tt * TN:(tt + 1) * TN],
                    start=True,
                    stop=True,
                )
                bc16 = bcpool.tile([128, TN], bf16, tag="bc")
                nc.scalar.copy(bc16[:], pb[:])
                scr = bcpool.tile([128, KC * TN], bf16, tag="scr", bufs=2)
                for kc in range(KC):
                    nc.vector.tensor_mul(
                        scr[:, kc * TN:(kc + 1) * TN],
                        kt[kc][:, tt * TN:(tt + 1) * TN],
                        bc16[:],
                    )
                ctxp = bcpool.tile([128, KC], f32, tag="ctxp", bufs=9)
                nc.vector.reduce_sum(
                    ctxp[:], scr[:].rearrange("p (kc t) -> p kc t", kc=KC), axis=AX.X
                )
                state[b][4].append(ctxp)

            def emit_finish(b):
                """normalize: attn = ex/sum out, ctx = (sum of partials)/sum"""
                kt, mask_sb, ex16, se4, ctxps = state.pop(b)
                se = p0pool.tile([1, 1], f32, tag="se")
                nc.vector.reduce_sum(se[:], se4[:], axis=AX.X)
                rcp = p0pool.tile([1, 1], f32, tag="rcp")
                nc.vector.reciprocal(rcp[:], se[:])
                attn_f = p0pool.tile([1, T], f32, tag="attnf")
                nc.vector.tensor_scalar_mul(attn_f[:], ex16[:], rcp[:])
                nc.scalar.dma_start(out=attn_p[b].unsqueeze(0), in_=attn_f[:])
                # rcp broadcast across partitions via fp32 rank-1 matmul
                prc = psum.tile([128, 1], f32, tag="pb", bufs=2)
                nc.tensor.matmul(prc[:], lhsT=ones_f[:], rhs=rcp[:], start=True, stop=True)
                rcp_bc = bcpool.tile([128, 1], f32, tag="rcpbc", bufs=2)
                nc.scalar.copy(rcp_bc[:], prc[:])
                acc = bcpool.tile([128, KC], f32, tag="ctxacc", bufs=2)
                nc.vector.tensor_add(acc[:], ctxps[0][:], ctxps[1][:])
                nc.vector.tensor_add(acc[:], acc[:], ctxps[2][:])
                nc.vector.tensor_add(acc[:], acc[:], ctxps[3][:])
                ctx_sb = bcpool.tile([128, KC], f32, tag="ctx", bufs=2)
                nc.vector.tensor_scalar_mul(ctx_sb[:], acc[:], rcp_bc[:])
                nc.scalar.dma_start(out=ctx_p[b], in_=ctx_sb[:])

            def start_batch(b):
                mask_sb = p0pool.tile([1, T], bf16, tag="mask")
                nc.scalar.dma_start(out=mask_sb[:], in_=maskb_p[b].unsqueeze(0))
                ex16 = p0pool.tile([1, T], bf16, tag="ex16")
                se4 = p0pool.tile([1, TT], f32, tag="se4")
                state[b] = [kt_tiles.pop(b), mask_sb, ex16, se4, []]

            # finish(b-1) is deferred past batch b's matmuls so its rcp
            # broadcast matmul never stalls the PE on the DVE softmax chain
            start_batch(0)
            for b in range(BPC):
                if b + 1 < BPC:
                    load_kt(b + 1)
                for tt in range(TT):
                    emit_tt(b, tt)
                    if tt > 0:
                        emit_ctx_tt(b, tt - 1)
                if b > 0:
                    emit_finish(b - 1)
                if b + 1 < BPC:
                    start_batch(b + 1)
                emit_ctx_tt(b, TT - 1)
            emit_finish(BPC - 1)

    nc.compile()
    return nc


def _prep_in_maps(queries, keys, pad_mask, W_q, W_k, w_v):
    """Host-side sharding + layout prep (dtype casts, transposes, mask bias)."""
    wk16 = np.ascontiguousarray(W_k.astype(BF16)).reshape(KC, 128, H)
    wq16 = np.ascontiguousarray(W_q.astype(BF16)).reshape(KC, 128, H)
    wv16 = np.ascontiguousarray(w_v[:, 0].reshape(HC, 128).T.astype(BF16))
    maskb = np.where(pad_mask[:, :, 0], np.float32(-1e9), np.float32(0.0)).astype(BF16)

    in_maps = []
    for c in range(NCORES):
        b0 = c * BPC
        kT = np.ascontiguousarray(
            keys[b0:b0 + BPC].transpose(0, 2, 1)
        ).astype(BF16).reshape(BPC, KC, 128, T)
        qt = np.ascontiguousarray(
            queries[b0:b0 + BPC, 0, :].T.reshape(KC, 128, BPC)
        ).astype(BF16)
        in_maps.append({
            "keysT": kT,
            "wk": wk16,
            "wq": wq16,
            "qt": qt,
            "wv": wv16,
            "maskb": np.ascontiguousarray(maskb[b0:b0 + BPC]),
        })
    return in_maps


def run(inputs, trace=False):
    """Build, compile, run on 8 cores. Returns ((attn, context), exec_time_ns)."""
    from concourse.bass_utils import run_bass_kernel_spmd

    in_maps = _prep_in_maps(**inputs)
    nc = build_nc()
    res = run_bass_kernel_spmd(nc, in_maps, list(range(NCORES)), trace=trace)

    attn = np.empty((B, T, 1), dtype=np.float32)
    context = np.empty((B, KD, 1), dtype=np.float32)
    for c in range(NCORES):
        b0 = c * BPC
        attn[b0:b0 + BPC, :, 0] = res.results[c]["attn"]
        # ctx result layout [b, p, kc] -> k = kc*128 + p
        context[b0:b0 + BPC, :, 0] = (
            res.results[c]["ctx"].transpose(0, 2, 1).reshape(BPC, KD)
        )
    return (attn, context), res.exec_time_ns


def kernel(queries, keys, pad_mask, W_q, W_k, w_v):
    (attn, context), _ = run(
        dict(queries=np.asarray(queries), keys=np.asarray(keys),
             pad_mask=np.asarray(pad_mask), W_q=np.asarray(W_q),
             W_k=np.asarray(W_k), w_v=np.asarray(w_v))
    )
    return attn, context
